# revision 9
# baseline (speedup 1.0000x reference)
"""Trainium2 Bass kernel for full-dim attention — bf16 fully-resident variant.

Folded algorithm (scores = x M x.T / sqrt(D) with M = wq.T wk;
out = (p x) W2.T with W2 = wo wv).  All matmul operands are bf16
(full-rate PE + FWL weight-load hiding + half the HBM traffic of the
fp32r variant), accumulation stays fp32 in PSUM, softmax statistics in
fp32, output written fp32.  Measured rel err ~4e-3 (tolerance 2e-2).

Everything is resident in SBUF (~174 KB/partition of 208), so x streams
from HBM exactly twice (transposed + natural layout), 12 MB input DMA
per core, and the whole SQ=1024 query block runs in a single pass.

DMA descriptor generation costs ~630 ns per dma_start, serialized on
the Sync engine, so inputs are batched into few large per-partition-
contiguous transfers; only the first uT sweep's operands (mTa + the
first xTq half) stay fine-grained (256 KB, interleaved in consumption
order) so the PE can start before the stream completes.
"""

import sys

if "/opt/trn_rl_repo" not in sys.path:
    sys.path.insert(0, "/opt/trn_rl_repo")

import numpy as np
import ml_dtypes

N_CORES = 8
P = 128

_BUILD_CACHE = {}


def _build(S, D, SQ):
    import concourse.mybir as mybir
    import concourse.tile as tile
    from concourse import bacc

    key = (S, D, SQ)
    if key in _BUILD_CACHE:
        return _BUILD_CACHE[key]

    dt = mybir.dt
    DS = D // P           # d subtiles (8)
    SK = S - SQ           # keys beyond the query block (1024)
    SKT = S // P          # key tiles (16)
    SQT = SQ // P         # query tiles (8)
    KT = SK // P          # non-query key tiles (8)
    NB = 512              # PSUM free-dim block (one fp32 bank)
    GB = D // NB          # output column blocks (2)
    QB = SQ // NB         # query column blocks (2)
    QTB = NB // P         # query tiles per column block (4)
    DTG = 4               # d'-tiles per uT sweep
    INV_SQRT_D = 1.0 / float(np.sqrt(np.float32(D)))

    nc = bacc.Bacc(None, target_bir_lowering=False, debug=False)

    bf = dt.bfloat16
    xTq_d = nc.dram_tensor("xTq", [P, QB, DS, NB], bf, kind="ExternalInput")
    xTk_d = nc.dram_tensor("xTk", [P, DS, SK], bf, kind="ExternalInput")
    xn_d = nc.dram_tensor("xn", [P, DS, SKT, P], bf, kind="ExternalInput")
    mTa_d = nc.dram_tensor("mTa", [P, DS, NB], bf, kind="ExternalInput")
    mTb_d = nc.dram_tensor("mTb", [P, DS, NB], bf, kind="ExternalInput")
    w2T_d = nc.dram_tensor("w2T", [P, DS, D], bf, kind="ExternalInput")
    y_d = nc.dram_tensor("y", [SQ, D], dt.float32, kind="ExternalOutput")

    with tile.TileContext(nc) as tc:
        with (
            tc.tile_pool(name="xTq", bufs=1) as xTq_pool,
            tc.tile_pool(name="xTk", bufs=1) as xTk_pool,
            tc.tile_pool(name="xn", bufs=1) as xn_pool,
            tc.tile_pool(name="mT", bufs=1) as mT_pool,
            tc.tile_pool(name="w2T", bufs=1) as w2T_pool,
            tc.tile_pool(name="uT", bufs=1) as uT_pool,
            tc.tile_pool(name="pT", bufs=1) as pT_pool,
            tc.tile_pool(name="px", bufs=1) as px_pool,
            tc.tile_pool(name="stat", bufs=1) as stat_pool,
            tc.tile_pool(name="outsb", bufs=4) as out_pool,
            tc.tile_pool(name="ps", bufs=7, space="PSUM") as ps_pool,
            tc.tile_pool(name="zps", bufs=1, space="PSUM") as z_pool,
        ):
            xTq = xTq_pool.tile([P, QB, DS, NB], bf)
            xTk = xTk_pool.tile([P, DS, SK], bf)
            xn = xn_pool.tile([P, DS, SKT, P], bf)
            mTa = mT_pool.tile([P, DS, NB], bf, name="mTa")
            mTb = mT_pool.tile([P, DS, NB], bf, name="mTb")
            w2T = w2T_pool.tile([P, DS, D], bf)

            S1 = stat_pool.tile([P, SQ], dt.float32)
            S1b = stat_pool.tile([P, SQ], bf, name="S1b")
            zs = stat_pool.tile([P, SQT], dt.float32, name="zs")
            ones = stat_pool.tile([P, 1], bf, name="ones")
            nc.vector.memset(ones[:], 1.0)

            # PE warmup: ~3.9us of matmuls to lift HAM to K=8/8 while the
            # first input chunks stream in
            wrm = stat_pool.tile([P, P], bf, name="wrm")
            nc.vector.memset(wrm[:], 0.0)
            wps = z_pool.tile([P, P], dt.float32, tag="zp", name="wps")
            for i in range(36):
                nc.tensor.matmul(wps[:], wrm[:], wrm[:], start=True, stop=True)

            # ---- input DMA, batched, in consumption order ----
            # first uT sweep's operands fine-grained (2-ds 256 KB chunks,
            # mTa/xTq interleaved), everything later as large single calls
            for ds in range(2):
                nc.sync.dma_start(mTa[:, ds:ds + 1, :], mTa_d[:, ds:ds + 1, :])
                nc.sync.dma_start(xTq[:, 0, ds:ds + 1, :],
                                  xTq_d[:, 0, ds:ds + 1, :])
            for dh in range(1, DS // 2):
                nc.sync.dma_start(mTa[:, 2 * dh:2 * dh + 2, :],
                                  mTa_d[:, 2 * dh:2 * dh + 2, :])
                nc.sync.dma_start(xTq[:, 0, 2 * dh:2 * dh + 2, :],
                                  xTq_d[:, 0, 2 * dh:2 * dh + 2, :])
            for dh in range(DS // 2):
                nc.sync.dma_start(xTq[:, 1, 2 * dh:2 * dh + 2, :],
                                  xTq_d[:, 1, 2 * dh:2 * dh + 2, :])
            nc.sync.dma_start(mTb[:], mTb_d[:])
            nc.sync.dma_start(xTk[:], xTk_d[:])
            nc.sync.dma_start(xn[:, :DS // 2, :, :], xn_d[:, :DS // 2, :, :])
            nc.sync.dma_start(xn[:, DS // 2:, :, :], xn_d[:, DS // 2:, :, :])
            nc.sync.dma_start(w2T[:], w2T_d[:])

            # ---- A: uT[d', sq] = sum_d mT[d, d'] xT[d, sq]  (u = x M) ----
            uT = uT_pool.tile([P, DS, SQ], bf)
            for g in range(DS // DTG):
                mT = mTa if g == 0 else mTb
                for qb in range(QB):
                    pss = [ps_pool.tile([P, NB], dt.float32, tag="ps",
                                        name=f"ps_u{g}_{qb}_{j}") for j in range(DTG)]
                    for ds in range(DS):
                        for j in range(DTG):
                            nc.tensor.matmul(
                                pss[j][:], mT[:, ds, j * P:(j + 1) * P],
                                xTq[:, qb, ds, :],
                                start=(ds == 0), stop=(ds == DS - 1),
                            )
                    for j in range(DTG):
                        dt_ = g * DTG + j
                        nc.any.tensor_copy(uT[:, dt_, qb * NB:(qb + 1) * NB], pss[j][:])

            # ---- B: pT[sk, sq] = exp(scores.T / sqrt(D)) ----
            pT = pT_pool.tile([P, SKT, SQ], bf)
            for skt in range(SKT):
                for qb in range(QB):
                    ps1 = ps_pool.tile([P, NB], dt.float32, tag="ps",
                                       name=f"ps_s{skt}_{qb}")
                    for ds in range(DS):
                        if skt < SQT:
                            lhs = xTq[:, skt // QTB, ds,
                                      (skt % QTB) * P:(skt % QTB + 1) * P]
                        else:
                            lhs = xTk[:, ds, (skt - SQT) * P:(skt - SQT + 1) * P]
                        nc.tensor.matmul(
                            ps1[:], lhs, uT[:, ds, qb * NB:(qb + 1) * NB],
                            start=(ds == 0), stop=(ds == DS - 1),
                        )
                    nc.scalar.activation(
                        pT[:, skt, qb * NB:(qb + 1) * NB], ps1[:],
                        mybir.ActivationFunctionType.Exp, scale=INV_SQRT_D,
                    )
                    dst = S1[:, qb * NB:(qb + 1) * NB]
                    if skt == 0:
                        nc.vector.tensor_copy(dst, pT[:, 0, qb * NB:(qb + 1) * NB])
                    else:
                        nc.vector.tensor_add(dst, dst,
                                             pT[:, skt, qb * NB:(qb + 1) * NB])

            # ---- D: px[d, sq] = sum_sk xn[sk, d] pT[sk, sq] ----
            # (Z reduction C is slotted in after dt_=0 so its tiny matmuls
            # never stall the PE on the softmax-statistics chain.)
            px = px_pool.tile([P, DS, SQ], bf)
            for dt_ in range(DS):
                for qb in range(QB):
                    ps2 = ps_pool.tile([P, NB], dt.float32, tag="ps",
                                       name=f"ps_c{dt_}_{qb}")
                    for skt in range(SKT):
                        nc.tensor.matmul(
                            ps2[:], xn[:, dt_, skt, :],
                            pT[:, skt, qb * NB:(qb + 1) * NB],
                            start=(skt == 0), stop=(skt == SKT - 1),
                        )
                    nc.any.tensor_copy(px[:, dt_, qb * NB:(qb + 1) * NB], ps2[:])

                if dt_ == 0:
                    # ---- C: Z and 1/Z ----
                    nc.vector.tensor_copy(S1b[:], S1[:])
                    zpc = z_pool.tile([P, SQT], dt.float32, tag="zp", name="zpc")
                    for t in range(SQT):
                        nc.tensor.matmul(zpc[:, t:t + 1], S1b[:, t * P:(t + 1) * P],
                                         ones[:, 0:1], start=True, stop=True)
                    nc.vector.reciprocal(zs[:], zpc[:])

            # ---- E: y[sq, g] = (sum_d px[d, sq] w2T[d, g]) / Z ----
            # gb-sequential so the first half's scale+store overlaps the
            # second half's matmuls; the last tile keeps split DMAs so the
            # final store chain after the last matmul is short.
            for t in range(SQT):
                ot = out_pool.tile([P, D], dt.float32, tag="ot", name=f"ot{t}")
                for gb in range(GB):
                    pso = ps_pool.tile([P, NB], dt.float32, tag="ps",
                                       name=f"ps_o{t}_{gb}")
                    for ds in range(DS):
                        nc.tensor.matmul(
                            pso[:], px[:, ds, t * P:(t + 1) * P],
                            w2T[:, ds, gb * NB:(gb + 1) * NB],
                            start=(ds == 0), stop=(ds == DS - 1),
                        )
                    nc.vector.tensor_mul(
                        ot[:, gb * NB:(gb + 1) * NB], pso[:],
                        zs[:, t:t + 1].to_broadcast([P, NB]))
                    if t == SQT - 1:
                        nc.sync.dma_start(
                            y_d[t * P:(t + 1) * P, gb * NB:(gb + 1) * NB],
                            ot[:, gb * NB:(gb + 1) * NB])
                if t < SQT - 1:
                    nc.sync.dma_start(y_d[t * P:(t + 1) * P, :], ot[:])

    nc.compile()
    _BUILD_CACHE[key] = nc
    return nc


def _run(x, wq, wk, wv, wo, trace=False):
    from concourse.bass_utils import run_bass_kernel_spmd

    B, S, D = x.shape
    SQ = B * S // N_CORES
    halves = S // SQ
    DS = D // P
    SKT = S // P
    NB = 512
    nc = _build(S, D, SQ)

    x = np.asarray(x, dtype=np.float32)
    wq = np.asarray(wq, dtype=np.float32)
    wk = np.asarray(wk, dtype=np.float32)
    wv = np.asarray(wv, dtype=np.float32)
    wo = np.asarray(wo, dtype=np.float32)
    M = wq.T @ wk
    W2 = wo @ wv

    def prep_T(a):
        # [k, n] -> bf16 [128, k/128, n] (k on partitions)
        k, n = a.shape
        return np.ascontiguousarray(
            a.reshape(k // P, P, n).transpose(1, 0, 2).astype(ml_dtypes.bfloat16))

    mTa = prep_T(M[:, :NB])
    mTb = prep_T(M[:, NB:])
    w2T = prep_T(np.ascontiguousarray(W2.T))

    in_maps = []
    for c in range(N_CORES):
        b, h = divmod(c, halves)
        xb = x[b]
        if h != 0:
            xb = np.concatenate([xb[h * SQ:(h + 1) * SQ], xb[:h * SQ],
                                 xb[(h + 1) * SQ:]], axis=0)
        xb16 = np.asarray(xb, dtype=ml_dtypes.bfloat16)
        # queries x.T, qb-major: [128, QB, DS, NB]
        xTq = np.ascontiguousarray(
            xb16[:SQ].T.reshape(DS, P, SQ // NB, NB).transpose(1, 2, 0, 3))
        # keys beyond the query block, x.T: [128, DS, SK]
        xTk = np.ascontiguousarray(
            xb16[SQ:].T.reshape(DS, P, S - SQ).transpose(1, 0, 2))
        # natural-x px lhsT tiles, key-within-tile on partitions:
        # [128, DS, SKT, 128], matching the SBUF tile exactly
        xn = np.ascontiguousarray(
            xb16.reshape(SKT, P, DS, P).transpose(1, 2, 0, 3))
        in_maps.append({"xTq": xTq, "xTk": xTk, "xn": xn,
                        "mTa": mTa, "mTb": mTb, "w2T": w2T})

    res = run_bass_kernel_spmd(nc, in_maps, core_ids=list(range(N_CORES)),
                               trace=trace)
    out = np.empty((B, S, D), dtype=np.float32)
    for c in range(N_CORES):
        b, h = divmod(c, halves)
        out[b, h * SQ:(h + 1) * SQ, :] = res.results[c]["y"]
    return out, res


def kernel(x, wq, wk, wv, wo):
    out, _ = _run(x, wq, wk, wv, wo)
    return out


# revision 10
# speedup vs baseline: 1.0147x; 1.0147x over previous
"""Trainium2 Bass kernel for full-dim attention — bf16 fully-resident variant.

Folded algorithm (scores = x M x.T / sqrt(D) with M = wq.T wk;
out = (p x) W2.T with W2 = wo wv).  All matmul operands are bf16
(full-rate PE + FWL weight-load hiding + half the HBM traffic of the
fp32r variant), accumulation stays fp32 in PSUM, softmax statistics in
fp32, output written fp32.  Measured rel err ~4e-3 (tolerance 2e-2).

Everything is resident in SBUF (~174 KB/partition of 208), so x streams
from HBM exactly twice (transposed + natural layout), 12 MB input DMA
per core, and the whole SQ=1024 query block runs in a single pass.

DMA descriptor generation costs ~630 ns per dma_start, serialized on
the Sync engine, so inputs are batched into few large per-partition-
contiguous transfers; only the first uT sweep's operands (mTa + the
first xTq half) stay fine-grained (256 KB, interleaved in consumption
order) so the PE can start before the stream completes.
"""

import sys

if "/opt/trn_rl_repo" not in sys.path:
    sys.path.insert(0, "/opt/trn_rl_repo")

import numpy as np
import ml_dtypes

N_CORES = 8
P = 128

_BUILD_CACHE = {}


def _build(S, D, SQ):
    import concourse.mybir as mybir
    import concourse.tile as tile
    from concourse import bacc

    key = (S, D, SQ)
    if key in _BUILD_CACHE:
        return _BUILD_CACHE[key]

    dt = mybir.dt
    DS = D // P           # d subtiles (8)
    SK = S - SQ           # keys beyond the query block (1024)
    SKT = S // P          # key tiles (16)
    SQT = SQ // P         # query tiles (8)
    KT = SK // P          # non-query key tiles (8)
    NB = 512              # PSUM free-dim block (one fp32 bank)
    GB = D // NB          # output column blocks (2)
    QB = SQ // NB         # query column blocks (2)
    QTB = NB // P         # query tiles per column block (4)
    DTG = 4               # d'-tiles per uT sweep
    INV_SQRT_D = 1.0 / float(np.sqrt(np.float32(D)))

    nc = bacc.Bacc(None, target_bir_lowering=False, debug=False)

    bf = dt.bfloat16
    xTq_d = nc.dram_tensor("xTq", [P, QB, DS, NB], bf, kind="ExternalInput")
    xTk_d = nc.dram_tensor("xTk", [P, DS, SK], bf, kind="ExternalInput")
    xn_d = nc.dram_tensor("xn", [P, DS, SKT, P], bf, kind="ExternalInput")
    mTa_d = nc.dram_tensor("mTa", [P, DS, NB], bf, kind="ExternalInput")
    mTb_d = nc.dram_tensor("mTb", [P, DS, NB], bf, kind="ExternalInput")
    w2T_d = nc.dram_tensor("w2T", [P, DS, D], bf, kind="ExternalInput")
    y_d = nc.dram_tensor("y", [SQ, D], dt.float32, kind="ExternalOutput")

    with tile.TileContext(nc) as tc:
        with (
            tc.tile_pool(name="xTq", bufs=1) as xTq_pool,
            tc.tile_pool(name="xTk", bufs=1) as xTk_pool,
            tc.tile_pool(name="xn", bufs=1) as xn_pool,
            tc.tile_pool(name="mT", bufs=1) as mT_pool,
            tc.tile_pool(name="w2T", bufs=1) as w2T_pool,
            tc.tile_pool(name="uT", bufs=1) as uT_pool,
            tc.tile_pool(name="pT", bufs=1) as pT_pool,
            tc.tile_pool(name="px", bufs=1) as px_pool,
            tc.tile_pool(name="stat", bufs=1) as stat_pool,
            tc.tile_pool(name="outsb", bufs=4) as out_pool,
            tc.tile_pool(name="ps", bufs=7, space="PSUM") as ps_pool,
            tc.tile_pool(name="zps", bufs=1, space="PSUM") as z_pool,
        ):
            xTq = xTq_pool.tile([P, QB, DS, NB], bf)
            xTk = xTk_pool.tile([P, DS, SK], bf)
            xn = xn_pool.tile([P, DS, SKT, P], bf)
            mTa = mT_pool.tile([P, DS, NB], bf, name="mTa")
            mTb = mT_pool.tile([P, DS, NB], bf, name="mTb")
            w2T = w2T_pool.tile([P, DS, D], bf)

            S1 = stat_pool.tile([P, SQ], dt.float32)
            S1b = stat_pool.tile([P, SQ], bf, name="S1b")
            zs = stat_pool.tile([P, SQT], dt.float32, name="zs")
            ones = stat_pool.tile([P, 1], bf, name="ones")
            nc.vector.memset(ones[:], 1.0)

            # PE warmup: ~3.9us of matmuls to lift HAM to K=8/8 while the
            # first input chunks stream in
            wrm = stat_pool.tile([P, P], bf, name="wrm")
            nc.vector.memset(wrm[:], 0.0)
            wps = z_pool.tile([P, P], dt.float32, tag="zp", name="wps")
            for i in range(36):
                nc.tensor.matmul(wps[:], wrm[:], wrm[:], start=True, stop=True)

            # ---- input DMA, batched, in consumption order ----
            # first uT sweep's operands fine-grained (2-ds 256 KB chunks,
            # mTa/xTq interleaved), everything later as large single calls
            for dh in range(DS // 2):
                nc.sync.dma_start(mTa[:, 2 * dh:2 * dh + 2, :],
                                  mTa_d[:, 2 * dh:2 * dh + 2, :])
                nc.sync.dma_start(xTq[:, 0, 2 * dh:2 * dh + 2, :],
                                  xTq_d[:, 0, 2 * dh:2 * dh + 2, :])
            for dh in range(DS // 2):
                nc.sync.dma_start(xTq[:, 1, 2 * dh:2 * dh + 2, :],
                                  xTq_d[:, 1, 2 * dh:2 * dh + 2, :])
            nc.sync.dma_start(mTb[:], mTb_d[:])
            nc.sync.dma_start(xTk[:], xTk_d[:])
            nc.sync.dma_start(xn[:, :DS // 2, :, :], xn_d[:, :DS // 2, :, :])
            nc.sync.dma_start(xn[:, DS // 2:, :, :], xn_d[:, DS // 2:, :, :])
            nc.sync.dma_start(w2T[:], w2T_d[:])

            # ---- A: uT[d', sq] = sum_d mT[d, d'] xT[d, sq]  (u = x M) ----
            uT = uT_pool.tile([P, DS, SQ], bf)
            for g in range(DS // DTG):
                mT = mTa if g == 0 else mTb
                for qb in range(QB):
                    pss = [ps_pool.tile([P, NB], dt.float32, tag="ps",
                                        name=f"ps_u{g}_{qb}_{j}") for j in range(DTG)]
                    for ds in range(DS):
                        for j in range(DTG):
                            nc.tensor.matmul(
                                pss[j][:], mT[:, ds, j * P:(j + 1) * P],
                                xTq[:, qb, ds, :],
                                start=(ds == 0), stop=(ds == DS - 1),
                            )
                    for j in range(DTG):
                        dt_ = g * DTG + j
                        nc.any.tensor_copy(uT[:, dt_, qb * NB:(qb + 1) * NB], pss[j][:])

            # ---- B: pT[sk, sq] = exp(scores.T / sqrt(D)) ----
            pT = pT_pool.tile([P, SKT, SQ], bf)
            for skt in range(SKT):
                for qb in range(QB):
                    ps1 = ps_pool.tile([P, NB], dt.float32, tag="ps",
                                       name=f"ps_s{skt}_{qb}")
                    for ds in range(DS):
                        if skt < SQT:
                            lhs = xTq[:, skt // QTB, ds,
                                      (skt % QTB) * P:(skt % QTB + 1) * P]
                        else:
                            lhs = xTk[:, ds, (skt - SQT) * P:(skt - SQT + 1) * P]
                        nc.tensor.matmul(
                            ps1[:], lhs, uT[:, ds, qb * NB:(qb + 1) * NB],
                            start=(ds == 0), stop=(ds == DS - 1),
                        )
                    nc.scalar.activation(
                        pT[:, skt, qb * NB:(qb + 1) * NB], ps1[:],
                        mybir.ActivationFunctionType.Exp, scale=INV_SQRT_D,
                    )
                    dst = S1[:, qb * NB:(qb + 1) * NB]
                    if skt == 0:
                        nc.vector.tensor_copy(dst, pT[:, 0, qb * NB:(qb + 1) * NB])
                    else:
                        nc.vector.tensor_add(dst, dst,
                                             pT[:, skt, qb * NB:(qb + 1) * NB])

            # ---- D: px[d, sq] = sum_sk xn[sk, d] pT[sk, sq] ----
            # (Z reduction C is slotted in after dt_=0 so its tiny matmuls
            # never stall the PE on the softmax-statistics chain.)
            px = px_pool.tile([P, DS, SQ], bf)
            for dt_ in range(DS):
                for qb in range(QB):
                    ps2 = ps_pool.tile([P, NB], dt.float32, tag="ps",
                                       name=f"ps_c{dt_}_{qb}")
                    for skt in range(SKT):
                        nc.tensor.matmul(
                            ps2[:], xn[:, dt_, skt, :],
                            pT[:, skt, qb * NB:(qb + 1) * NB],
                            start=(skt == 0), stop=(skt == SKT - 1),
                        )
                    nc.any.tensor_copy(px[:, dt_, qb * NB:(qb + 1) * NB], ps2[:])

                if dt_ == 0:
                    # ---- C: Z and 1/Z ----
                    nc.vector.tensor_copy(S1b[:], S1[:])
                    zpc = z_pool.tile([P, SQT], dt.float32, tag="zp", name="zpc")
                    for t in range(SQT):
                        nc.tensor.matmul(zpc[:, t:t + 1], S1b[:, t * P:(t + 1) * P],
                                         ones[:, 0:1], start=True, stop=True)
                    nc.vector.reciprocal(zs[:], zpc[:])

            # ---- E: y[sq, g] = (sum_d px[d, sq] w2T[d, g]) / Z ----
            # gb-sequential so the first half's scale+store overlaps the
            # second half's matmuls; the last tile keeps split DMAs so the
            # final store chain after the last matmul is short.
            for t in range(SQT):
                ot = out_pool.tile([P, D], dt.float32, tag="ot", name=f"ot{t}")
                for gb in range(GB):
                    pso = ps_pool.tile([P, NB], dt.float32, tag="ps",
                                       name=f"ps_o{t}_{gb}")
                    for ds in range(DS):
                        nc.tensor.matmul(
                            pso[:], px[:, ds, t * P:(t + 1) * P],
                            w2T[:, ds, gb * NB:(gb + 1) * NB],
                            start=(ds == 0), stop=(ds == DS - 1),
                        )
                    nc.vector.tensor_mul(
                        ot[:, gb * NB:(gb + 1) * NB], pso[:],
                        zs[:, t:t + 1].to_broadcast([P, NB]))
                    if t == SQT - 1:
                        nc.sync.dma_start(
                            y_d[t * P:(t + 1) * P, gb * NB:(gb + 1) * NB],
                            ot[:, gb * NB:(gb + 1) * NB])
                if t < SQT - 1:
                    nc.sync.dma_start(y_d[t * P:(t + 1) * P, :], ot[:])

    nc.compile()
    _BUILD_CACHE[key] = nc
    return nc


def _run(x, wq, wk, wv, wo, trace=False):
    from concourse.bass_utils import run_bass_kernel_spmd

    B, S, D = x.shape
    SQ = B * S // N_CORES
    halves = S // SQ
    DS = D // P
    SKT = S // P
    NB = 512
    nc = _build(S, D, SQ)

    x = np.asarray(x, dtype=np.float32)
    wq = np.asarray(wq, dtype=np.float32)
    wk = np.asarray(wk, dtype=np.float32)
    wv = np.asarray(wv, dtype=np.float32)
    wo = np.asarray(wo, dtype=np.float32)
    M = wq.T @ wk
    W2 = wo @ wv

    def prep_T(a):
        # [k, n] -> bf16 [128, k/128, n] (k on partitions)
        k, n = a.shape
        return np.ascontiguousarray(
            a.reshape(k // P, P, n).transpose(1, 0, 2).astype(ml_dtypes.bfloat16))

    mTa = prep_T(M[:, :NB])
    mTb = prep_T(M[:, NB:])
    w2T = prep_T(np.ascontiguousarray(W2.T))

    in_maps = []
    for c in range(N_CORES):
        b, h = divmod(c, halves)
        xb = x[b]
        if h != 0:
            xb = np.concatenate([xb[h * SQ:(h + 1) * SQ], xb[:h * SQ],
                                 xb[(h + 1) * SQ:]], axis=0)
        xb16 = np.asarray(xb, dtype=ml_dtypes.bfloat16)
        # queries x.T, qb-major: [128, QB, DS, NB]
        xTq = np.ascontiguousarray(
            xb16[:SQ].T.reshape(DS, P, SQ // NB, NB).transpose(1, 2, 0, 3))
        # keys beyond the query block, x.T: [128, DS, SK]
        xTk = np.ascontiguousarray(
            xb16[SQ:].T.reshape(DS, P, S - SQ).transpose(1, 0, 2))
        # natural-x px lhsT tiles, key-within-tile on partitions:
        # [128, DS, SKT, 128], matching the SBUF tile exactly
        xn = np.ascontiguousarray(
            xb16.reshape(SKT, P, DS, P).transpose(1, 2, 0, 3))
        in_maps.append({"xTq": xTq, "xTk": xTk, "xn": xn,
                        "mTa": mTa, "mTb": mTb, "w2T": w2T})

    res = run_bass_kernel_spmd(nc, in_maps, core_ids=list(range(N_CORES)),
                               trace=trace)
    out = np.empty((B, S, D), dtype=np.float32)
    for c in range(N_CORES):
        b, h = divmod(c, halves)
        out[b, h * SQ:(h + 1) * SQ, :] = res.results[c]["y"]
    return out, res


def kernel(x, wq, wk, wv, wo):
    out, _ = _run(x, wq, wk, wv, wo)
    return out
